# revision 6
# baseline (speedup 1.0000x reference)
"""AR(24) extrapolation kernel for Trainium2 (8 NeuronCores, data parallel).

The reference runs a 168-step scalar-weight autoregressive recurrence over the
last 24 timesteps of x, independently per (batch, channel).  Because the
recurrence is linear, output step t is a fixed linear combination of the
initial 24-sample window plus a bias term:

    y[b, t, d] = sum_i C[i, t] * x[b, S-24+i, d] + beta[t]

C [24, 168] and beta [168] follow from W/b by unrolling the recurrence once on
the host (float64, ~4k flops).  The device work is then a memory-bound
broadcast matmul: per core, out[t, (b, d)] = CB^T @ xaug where CB stacks
[C; beta] and xaug stacks [x_window^T; ones].

Sharding: pure data parallelism over batch (32 batches per core).  Device
output layout is [T, B_loc*D] so DMA stores have 16KB-contiguous runs per
partition; the host transposes back when gathering.
"""

import numpy as np

import concourse.bass as bass
import concourse.bacc as bacc
import concourse.tile as tile
from concourse import mybir
from concourse.bass_utils import run_bass_kernel_spmd

ORDER = 24
T = 168
D = 512
B = 256
S = 336
N_CORES = 8
NB = B // N_CORES        # 32 local batches per core
COLS = NB * D            # 16384 columns per core
GRP = 8                  # batches per staged output group
NGRP = NB // GRP         # 4 groups
P0 = 128                 # first t-chunk (partitions)
P1 = T - P0              # 40
F32 = mybir.dt.float32

_nc_cache = None


def _build_program():
    nc = bacc.Bacc()
    xaug = nc.declare_dram_parameter("xaug", [ORDER + 1, COLS], F32, isOutput=False)
    cb = nc.declare_dram_parameter("cb", [ORDER + 1, T], F32, isOutput=False)
    out = nc.declare_dram_parameter("out", [T, COLS], F32, isOutput=True)

    with tile.TileContext(nc) as tc:
        with (
            tc.tile_pool(name="consts", bufs=1) as consts,
            tc.tile_pool(name="xin", bufs=2) as xin,
            tc.tile_pool(name="stage", bufs=2) as stage,
            tc.tile_pool(name="psum", bufs=4, space="PSUM") as psum,
        ):
            cb_t = consts.tile([ORDER + 1, T], F32)
            nc.sync.dma_start(out=cb_t, in_=cb[:, :])

            for g in range(NGRP):
                c0 = g * GRP * D
                xg = xin.tile([ORDER + 1, GRP * D], F32, tag="xg")
                nc.sync.dma_start(out=xg, in_=xaug[:, c0 : c0 + GRP * D])

                st0 = stage.tile([P0, GRP * D], F32, tag="st0")
                st1 = stage.tile([P1, GRP * D], F32, tag="st1")
                for j in range(GRP):
                    mv = xg[:, j * D : (j + 1) * D]
                    ps0 = psum.tile([P0, D], F32, tag="ps0")
                    nc.tensor.matmul(ps0, cb_t[:, 0:P0], mv, start=True, stop=True)
                    nc.vector.tensor_copy(st0[:, j * D : (j + 1) * D], ps0)

                    ps1 = psum.tile([P1, D], F32, tag="ps1")
                    nc.tensor.matmul(ps1, cb_t[:, P0:T], mv, start=True, stop=True)
                    nc.vector.tensor_copy(st1[:, j * D : (j + 1) * D], ps1)

                nc.sync.dma_start(out=out[0:P0, c0 : c0 + GRP * D], in_=st0)
                nc.sync.dma_start(out=out[P0:T, c0 : c0 + GRP * D], in_=st1)

    nc.finalize()
    return nc


def _unroll_coeffs(W: np.ndarray, b: np.ndarray) -> np.ndarray:
    """Unroll the linear AR recurrence: CB[k, t] with rows 0..23 = window
    coefficients, row 24 = additive bias per step."""
    w = W[:, 0].astype(np.float64)
    bb = float(np.asarray(b).reshape(-1)[0])
    M = np.eye(ORDER)
    m = np.zeros(ORDER)
    CB = np.zeros((ORDER + 1, T), np.float64)
    for t in range(T):
        c = M.T @ w
        yb = m @ w + bb
        CB[:ORDER, t] = c
        CB[ORDER, t] = yb
        M = np.vstack([M[1:], c[None, :]])
        m = np.concatenate([m[1:], [yb]])
    return CB.astype(np.float32)


def kernel(x, W, b, tar_seq_len):
    global _nc_cache
    x = np.asarray(x, dtype=np.float32)
    W = np.asarray(W, dtype=np.float32)
    b = np.asarray(b, dtype=np.float32)
    assert int(tar_seq_len) == T, f"compiled for tar_seq_len={T}"
    assert x.shape == (B, S, D)

    CB = _unroll_coeffs(W, b)

    # host prep: last-24 window, transposed to [24, B*D], plus a ones row for
    # the bias term (so bias rides in the matmul contraction)
    xT = np.ascontiguousarray(x[:, -ORDER:, :].transpose(1, 0, 2)).reshape(
        ORDER, B * D
    )
    xaug = np.empty((ORDER + 1, B * D), np.float32)
    xaug[:ORDER] = xT
    xaug[ORDER] = 1.0

    if _nc_cache is None:
        _nc_cache = _build_program()
    nc = _nc_cache

    in_maps = [
        {
            "xaug": np.ascontiguousarray(xaug[:, c * COLS : (c + 1) * COLS]),
            "cb": CB,
        }
        for c in range(N_CORES)
    ]
    res = run_bass_kernel_spmd(nc, in_maps, list(range(N_CORES)))

    # gather: per-core [T, NB*D] -> [NB, T, D], concat over cores -> [B, T, D]
    parts = [
        r["out"].reshape(T, NB, D).transpose(1, 0, 2) for r in res.results
    ]
    return np.ascontiguousarray(np.concatenate(parts, axis=0))


# revision 10
# speedup vs baseline: 1.7846x; 1.7846x over previous
"""AR(24) extrapolation kernel for Trainium2 (8 NeuronCores, data parallel).

The reference runs a 168-step scalar-weight autoregressive recurrence over the
last 24 timesteps of x, independently per (batch, channel).  Because the
recurrence is linear, output step t is a fixed linear combination of the
initial 24-sample window plus a bias term:

    y[b, t, d] = sum_i C[i, t] * x[b, S-24+i, d] + beta[t]

C [24, 168] and beta [168] follow from W/b by unrolling the recurrence once on
the host (float64, ~4k flops).  The device work is then a memory-bound
broadcast matmul: per core, out[t, (b, d)] = CB^T @ xaug where CB stacks
[C; beta] and xaug stacks [x_window^T; ones].

Sharding: pure data parallelism over batch (32 batches per core).  Device
output layout is [T, B_loc*D] so DMA stores have 16KB-contiguous runs per
partition; the host transposes back when gathering.
"""

import numpy as np

import concourse.bass as bass
import concourse.bacc as bacc
import concourse.tile as tile
from concourse import mybir
from concourse.bass_utils import run_bass_kernel_spmd

ORDER = 24
T = 168
D = 512
B = 256
S = 336
N_CORES = 8
NB = B // N_CORES        # 32 local batches per core
COLS = NB * D            # 16384 columns per core
GRP = 8                  # batches per staged output group
NGRP = NB // GRP         # 4 groups
P0 = 128                 # first t-chunk (partitions)
P1 = T - P0              # 40
F32 = mybir.dt.float32
F32R = mybir.dt.float32r  # fast fp32 matmul path (full PE rate at N>=256)

_nc_cache = None


def _build_program():
    nc = bacc.Bacc()
    xaug = nc.declare_dram_parameter("xaug", [ORDER + 1, COLS], F32R, isOutput=False)
    cb = nc.declare_dram_parameter("cb", [ORDER + 1, T], F32R, isOutput=False)
    out = nc.declare_dram_parameter("out", [T, COLS], F32, isOutput=True)

    with tile.TileContext(nc) as tc:
        with (
            tc.tile_pool(name="consts", bufs=1) as consts,
            tc.tile_pool(name="xin", bufs=2) as xin,
            tc.tile_pool(name="stage", bufs=2) as stage,
            tc.tile_pool(name="psum", bufs=4, space="PSUM") as psum,
        ):
            cb_t = consts.tile([ORDER + 1, T], F32R)
            nc.sync.dma_start(out=cb_t, in_=cb[:, :])

            for g in range(NGRP):
                c0 = g * GRP * D
                xg = xin.tile([ORDER + 1, GRP * D], F32R, tag="xg")
                nc.sync.dma_start(out=xg, in_=xaug[:, c0 : c0 + GRP * D])

                st0 = stage.tile([P0, GRP * D], F32, tag="st0")
                st1 = stage.tile([P1, GRP * D], F32, tag="st1")
                for j in range(GRP):
                    mv = xg[:, j * D : (j + 1) * D]
                    ps0 = psum.tile([P0, D], F32, tag="ps0")
                    nc.tensor.matmul(ps0, cb_t[:, 0:P0], mv, start=True, stop=True)
                    nc.vector.tensor_copy(st0[:, j * D : (j + 1) * D], ps0)

                    ps1 = psum.tile([P1, D], F32, tag="ps1")
                    nc.tensor.matmul(ps1, cb_t[:, P0:T], mv, start=True, stop=True)
                    nc.vector.tensor_copy(st1[:, j * D : (j + 1) * D], ps1)

                nc.sync.dma_start(out=out[0:P0, c0 : c0 + GRP * D], in_=st0)
                nc.sync.dma_start(out=out[P0:T, c0 : c0 + GRP * D], in_=st1)

    nc.finalize()
    return nc


def _unroll_coeffs(W: np.ndarray, b: np.ndarray) -> np.ndarray:
    """Unroll the linear AR recurrence: CB[k, t] with rows 0..23 = window
    coefficients, row 24 = additive bias per step."""
    w = W[:, 0].astype(np.float64)
    bb = float(np.asarray(b).reshape(-1)[0])
    M = np.eye(ORDER)
    m = np.zeros(ORDER)
    CB = np.zeros((ORDER + 1, T), np.float64)
    for t in range(T):
        c = M.T @ w
        yb = m @ w + bb
        CB[:ORDER, t] = c
        CB[ORDER, t] = yb
        M = np.vstack([M[1:], c[None, :]])
        m = np.concatenate([m[1:], [yb]])
    return CB.astype(np.float32)


def kernel(x, W, b, tar_seq_len):
    global _nc_cache
    x = np.asarray(x, dtype=np.float32)
    W = np.asarray(W, dtype=np.float32)
    b = np.asarray(b, dtype=np.float32)
    assert int(tar_seq_len) == T, f"compiled for tar_seq_len={T}"
    assert x.shape == (B, S, D)

    CB = _unroll_coeffs(W, b)

    # host prep: last-24 window, transposed to [24, B*D], plus a ones row for
    # the bias term (so bias rides in the matmul contraction)
    xT = np.ascontiguousarray(x[:, -ORDER:, :].transpose(1, 0, 2)).reshape(
        ORDER, B * D
    )
    xaug = np.empty((ORDER + 1, B * D), np.float32)
    xaug[:ORDER] = xT
    xaug[ORDER] = 1.0

    if _nc_cache is None:
        _nc_cache = _build_program()
    nc = _nc_cache

    in_maps = [
        {
            "xaug": np.ascontiguousarray(xaug[:, c * COLS : (c + 1) * COLS]),
            "cb": CB,
        }
        for c in range(N_CORES)
    ]
    res = run_bass_kernel_spmd(nc, in_maps, list(range(N_CORES)))

    # gather: per-core [T, NB*D] -> [NB, T, D], concat over cores -> [B, T, D]
    parts = [
        r["out"].reshape(T, NB, D).transpose(1, 0, 2) for r in res.results
    ]
    return np.ascontiguousarray(np.concatenate(parts, axis=0))


# revision 11
# speedup vs baseline: 2.1487x; 1.2040x over previous
"""AR(24) extrapolation kernel for Trainium2 (8 NeuronCores, data parallel).

The reference runs a 168-step scalar-weight autoregressive recurrence over the
last 24 timesteps of x, independently per (batch, channel).  Because the
recurrence is linear, output step t is a fixed linear combination of the
initial 24-sample window plus a bias term:

    y[b, t, d] = sum_i C[i, t] * x[b, S-24+i, d] + beta[t]

C [24, 168] and beta [168] follow from W/b by unrolling the recurrence once on
the host (float64, ~4k flops).  The device work is then a memory-bound
broadcast matmul: per core, out[t, (b, d)] = CB^T @ xaug where CB stacks
[C; beta] and xaug stacks [x_window^T; ones].

Sharding: pure data parallelism over batch (32 batches per core).  Device
output layout is [T, B_loc*D] so DMA stores have 16KB-contiguous runs per
partition; the host transposes back when gathering.
"""

import numpy as np

import concourse.bass as bass
import concourse.bacc as bacc
import concourse.tile as tile
from concourse import mybir
from concourse.bass_utils import run_bass_kernel_spmd

ORDER = 24
T = 168
D = 512
B = 256
S = 336
N_CORES = 8
NB = B // N_CORES        # 32 local batches per core
COLS = NB * D            # 16384 columns per core
GRP = 8                  # batches per staged output group
NGRP = NB // GRP         # 4 groups
P0 = 128                 # first t-chunk (partitions)
P1 = T - P0              # 40
F32 = mybir.dt.float32
F32R = mybir.dt.float32r  # fast fp32 matmul path (full PE rate at N>=256)

_nc_cache = None


def _build_program():
    nc = bacc.Bacc()
    xaug = nc.declare_dram_parameter("xaug", [ORDER + 1, COLS], F32R, isOutput=False)
    cb = nc.declare_dram_parameter("cb", [ORDER + 1, T], F32R, isOutput=False)
    out = nc.declare_dram_parameter("out", [T, COLS], F32, isOutput=True)

    with tile.TileContext(nc) as tc:
        with (
            tc.tile_pool(name="consts", bufs=1) as consts,
            tc.tile_pool(name="xin", bufs=1) as xin,
            tc.tile_pool(name="stage", bufs=2) as stage,
            tc.tile_pool(name="psum", bufs=4, space="PSUM") as psum,
        ):
            # all input loads ride the (otherwise idle) gpsimd SWDGE ring,
            # keeping both HWDGE rings free for output stores
            cb_t = consts.tile([ORDER + 1, T], F32R)
            nc.gpsimd.dma_start(out=cb_t, in_=cb[:, :])
            xgs = []
            for g in range(NGRP):
                xg = xin.tile([ORDER + 1, GRP * D], F32R, tag=f"xg{g}")
                nc.gpsimd.dma_start(
                    out=xg, in_=xaug[:, g * GRP * D : (g + 1) * GRP * D]
                )
                xgs.append(xg)

            for g in range(NGRP):
                c0 = g * GRP * D
                st0 = stage.tile([P0, GRP * D], F32, tag="st0")
                st1 = stage.tile([P1, GRP * D], F32, tag="st1")
                for j in range(GRP):
                    mv = xgs[g][:, j * D : (j + 1) * D]
                    ps0 = psum.tile([P0, D], F32, tag="ps0")
                    nc.tensor.matmul(ps0, cb_t[:, 0:P0], mv, start=True, stop=True)
                    nc.vector.tensor_copy(st0[:, j * D : (j + 1) * D], ps0)

                    ps1 = psum.tile([P1, D], F32, tag="ps1")
                    nc.tensor.matmul(ps1, cb_t[:, P0:T], mv, start=True, stop=True)
                    nc.scalar.copy(st1[:, j * D : (j + 1) * D], ps1)

                # alternate the two HWDGE rings (SP / Activation) per group so
                # store bandwidth and completion latency overlap
                eng0, eng1 = (nc.sync, nc.scalar) if g % 2 == 0 else (nc.scalar, nc.sync)
                eng0.dma_start(out=out[0:P0, c0 : c0 + GRP * D], in_=st0)
                eng1.dma_start(out=out[P0:T, c0 : c0 + GRP * D], in_=st1)

    nc.finalize()
    return nc


def _unroll_coeffs(W: np.ndarray, b: np.ndarray) -> np.ndarray:
    """Unroll the linear AR recurrence: CB[k, t] with rows 0..23 = window
    coefficients, row 24 = additive bias per step."""
    w = W[:, 0].astype(np.float64)
    bb = float(np.asarray(b).reshape(-1)[0])
    M = np.eye(ORDER)
    m = np.zeros(ORDER)
    CB = np.zeros((ORDER + 1, T), np.float64)
    for t in range(T):
        c = M.T @ w
        yb = m @ w + bb
        CB[:ORDER, t] = c
        CB[ORDER, t] = yb
        M = np.vstack([M[1:], c[None, :]])
        m = np.concatenate([m[1:], [yb]])
    return CB.astype(np.float32)


def kernel(x, W, b, tar_seq_len):
    global _nc_cache
    x = np.asarray(x, dtype=np.float32)
    W = np.asarray(W, dtype=np.float32)
    b = np.asarray(b, dtype=np.float32)
    assert int(tar_seq_len) == T, f"compiled for tar_seq_len={T}"
    assert x.shape == (B, S, D)

    CB = _unroll_coeffs(W, b)

    # host prep: last-24 window, transposed to [24, B*D], plus a ones row for
    # the bias term (so bias rides in the matmul contraction)
    xT = np.ascontiguousarray(x[:, -ORDER:, :].transpose(1, 0, 2)).reshape(
        ORDER, B * D
    )
    xaug = np.empty((ORDER + 1, B * D), np.float32)
    xaug[:ORDER] = xT
    xaug[ORDER] = 1.0

    if _nc_cache is None:
        _nc_cache = _build_program()
    nc = _nc_cache

    in_maps = [
        {
            "xaug": np.ascontiguousarray(xaug[:, c * COLS : (c + 1) * COLS]),
            "cb": CB,
        }
        for c in range(N_CORES)
    ]
    res = run_bass_kernel_spmd(nc, in_maps, list(range(N_CORES)))

    # gather: per-core [T, NB*D] -> [NB, T, D], concat over cores -> [B, T, D]
    parts = [
        r["out"].reshape(T, NB, D).transpose(1, 0, 2) for r in res.results
    ]
    return np.ascontiguousarray(np.concatenate(parts, axis=0))
